# revision 1
# baseline (speedup 1.0000x reference)
"""Trainium2 Bass kernel for nn_CrossAttn (linear cross-attention, B=8 N=4096 C=1024 H=16).

Strategy:
  - Data-parallel over B across the 8 NeuronCores (batch-local math, no collectives).
  - Host pre-transposes activations to x^T [C, N] (C on partitions) and casts matmul
    operands to bf16; PSUM accumulation is fp32.
  - Self stage per stream: qkv GEMMs from x^T; linear ctx = softmax_d(k^T v * scale)
    accumulated as paired v^T k matmuls (2 heads -> one 128x128 MM); softmax along the
    free axis; ctx transposed via PE into a block-diagonal 2-head tile so the output
    product q @ ctx becomes (ctx_bd)^T @ q^T with K=128, N=512.
  - Cross stage: q is the post-self activation itself (already resident transposed);
    kv GEMMs with Wkv1/Wkv2; o1 = ctx2-product + x1', o2 = ctx1-product + x2'.
  - Outputs written transposed [C, N] fp32; host un-transposes.
"""

import os
import sys

sys.path.insert(0, "/opt/trn_rl_repo")

import numpy as np
import ml_dtypes

import concourse.bass as bass
import concourse.mybir as mybir
import concourse.tile as tile
from concourse import bacc
from concourse.masks import make_identity
from concourse.bass_utils import run_bass_kernel_spmd

B, N, C, H = 8, 4096, 1024, 16
D = C // H                 # 64
SCALE = D ** -0.5          # 0.125
P = 128                    # partitions
KT = C // P                # 8 contraction tiles
NT = N // P                # 32 n-tiles (ctx accumulation)
CH = N // 512              # 8 n-chunks of 512
PAIRS = H // 2             # 8 head pairs
F32 = mybir.dt.float32
BF16 = mybir.dt.bfloat16

_CACHE = {}


def _build():
    nc = bacc.Bacc(None, target_bir_lowering=False)

    x1T_d = nc.dram_tensor("x1T", [C, N], BF16, kind="ExternalInput")
    x2T_d = nc.dram_tensor("x2T", [C, N], BF16, kind="ExternalInput")
    Wsqkv_d = nc.dram_tensor("Wsqkv", [C, 3 * C], BF16, kind="ExternalInput")
    Wkv1_d = nc.dram_tensor("Wkv1", [C, 2 * C], BF16, kind="ExternalInput")
    Wkv2_d = nc.dram_tensor("Wkv2", [C, 2 * C], BF16, kind="ExternalInput")
    o1T_d = nc.dram_tensor("o1T", [C, N], F32, kind="ExternalOutput")
    o2T_d = nc.dram_tensor("o2T", [C, N], F32, kind="ExternalOutput")
    x1p_scr = nc.dram_tensor("x1p_scratch", [C, N], BF16, kind="Internal")

    # (kt*128 + p, n) -> [p, kt, n] view for per-partition-tile DMA
    x1T_r = x1T_d[:].rearrange("(t p) n -> p t n", p=P)
    x2T_r = x2T_d[:].rearrange("(t p) n -> p t n", p=P)
    Wsq_r = Wsqkv_d[:].rearrange("(t p) c -> p t c", p=P)
    Wkv1_r = Wkv1_d[:].rearrange("(t p) c -> p t c", p=P)
    Wkv2_r = Wkv2_d[:].rearrange("(t p) c -> p t c", p=P)
    o1T_r = o1T_d[:].rearrange("(t p) n -> p t n", p=P)
    o2T_r = o2T_d[:].rearrange("(t p) n -> p t n", p=P)
    x1p_r = x1p_scr[:].rearrange("(t p) n -> p t n", p=P)

    with tile.TileContext(nc) as tc:
        with (
            tc.tile_pool(name="xbig", bufs=2) as xbig,
            tc.tile_pool(name="wts", bufs=1) as wts,
            tc.tile_pool(name="kvsb", bufs=2) as kvsb,
            tc.tile_pool(name="qts", bufs=3) as qtsp,
            tc.tile_pool(name="ctxsb", bufs=2) as ctxsb,
            tc.tile_pool(name="ctxacc", bufs=1) as ctxaccp,
            tc.tile_pool(name="smax", bufs=2) as smaxp,
            tc.tile_pool(name="stats", bufs=4) as stats,
            tc.tile_pool(name="outst", bufs=2) as outst,
            tc.tile_pool(name="singles", bufs=1) as singles,
            tc.tile_pool(name="ps_kv", bufs=2, space="PSUM") as ps_kv,
            tc.tile_pool(name="ps_ctx", bufs=1, space="PSUM") as ps_ctx,
            tc.tile_pool(name="ps_qt", bufs=2, space="PSUM") as ps_qt,
            tc.tile_pool(name="ps_out", bufs=2, space="PSUM") as ps_out,
        ):
            ident = singles.tile([P, P], F32)
            make_identity(nc, ident)

            def ctx_accumulate(xt, W, kvcol0):
                """Accumulate per-pair ctx_rawT = v^T k over all n-tiles.

                xt: [P, KT, N] bf16 (activation transposed), W: [P, KT, wcols] bf16.
                Returns SBUF tile [P, PAIRS*128] fp32: pair p cols [128p,128p+128),
                head 2p block at rows 0:64 cols +0:64, head 2p+1 at rows 64:128
                cols +64:128 (off-diagonal blocks are garbage, never read).

                Accumulation across n-tiles happens in SBUF via DVE adds — four
                per-pair PSUM groups would share a bank, and each group's
                start=True clears has_written for the WHOLE bank, corrupting the
                other pairs' accumulation.
                """
                ctx_acc = ctxaccp.tile([P, PAIRS * P], F32, tag="ctxacc")

                def pair_mms(kv, nt):
                    ctx_ps = ps_ctx.tile([P, PAIRS * P], F32, tag="ctx")
                    for p in range(PAIRS):
                        nc.tensor.matmul(
                            ctx_ps[:, p * P:(p + 1) * P],
                            lhsT=kv[:, C + p * P: C + (p + 1) * P],   # v pair
                            rhs=kv[:, p * P:(p + 1) * P],             # k pair
                            start=True, stop=True,
                        )
                    if nt == 0:
                        nc.vector.tensor_copy(ctx_acc, ctx_ps)
                    else:
                        nc.vector.tensor_add(ctx_acc, ctx_acc, ctx_ps)

                prev = None
                for nt in range(NT):
                    kv = kvsb.tile([P, 2 * C], BF16, tag="kv")
                    for ch in range(4):
                        kv_ps = ps_kv.tile([P, 512], F32, tag="kvps")
                        for kt in range(KT):
                            nc.tensor.matmul(
                                kv_ps,
                                lhsT=xt[:, kt, nt * P:(nt + 1) * P],
                                rhs=W[:, kt, kvcol0 + ch * 512: kvcol0 + (ch + 1) * 512],
                                start=(kt == 0), stop=(kt == KT - 1),
                            )
                        nc.vector.tensor_copy(kv[:, ch * 512:(ch + 1) * 512], kv_ps)
                    if prev is not None:
                        pair_mms(*prev)
                    prev = (kv, nt)
                pair_mms(*prev)
                return ctx_acc

            def softmax_pair(ctx_ps, p, ctx_bd):
                """Softmax over d (free axis) of the two diag blocks of pair p, then
                PE-transpose into slice p of the block-diagonal bf16 ctx tile."""
                S = smaxp.tile([P, P], F32, tag="smax")
                nc.vector.memset(S, 0.0)
                for r0 in (0, 64):
                    blk = ctx_ps[r0:r0 + 64, p * P + r0: p * P + r0 + 64]
                    mx = stats.tile([P, 1], F32, tag="mx")
                    nc.vector.reduce_max(mx[r0:r0 + 64], blk, axis=mybir.AxisListType.X)
                    ng = stats.tile([P, 1], F32, tag="ng")
                    nc.scalar.mul(ng[r0:r0 + 64], mx[r0:r0 + 64], -SCALE)
                    se = stats.tile([P, 1], F32, tag="se")
                    nc.scalar.activation(
                        S[r0:r0 + 64, r0:r0 + 64], blk,
                        mybir.ActivationFunctionType.Exp,
                        bias=ng[r0:r0 + 64], scale=SCALE,
                        accum_out=se[r0:r0 + 64],
                    )
                    rv = stats.tile([P, 1], F32, tag="rv")
                    nc.vector.reciprocal(rv[r0:r0 + 64], se[r0:r0 + 64])
                    nc.vector.tensor_scalar_mul(
                        S[r0:r0 + 64, r0:r0 + 64], S[r0:r0 + 64, r0:r0 + 64],
                        rv[r0:r0 + 64],
                    )
                tr_ps = ps_out.tile([P, P], F32, tag="psout")
                nc.tensor.transpose(tr_ps, S, ident)
                nc.vector.tensor_copy(ctx_bd[:, p, :], tr_ps)

            def self_stage(xt, W, xp_out):
                """One self-attention branch: returns nothing; writes x' (bf16,
                transposed) into xp_out [P, KT, N]."""
                ctx_ps = ctx_accumulate(xt, W, kvcol0=C)
                ctx_bd = ctxsb.tile([P, PAIRS, P], BF16, tag="ctx_bd")
                for p in range(PAIRS):
                    softmax_pair(ctx_ps, p, ctx_bd)
                    for ch in range(CH):
                        qt_ps = ps_qt.tile([P, 512], F32, tag="qt")
                        for kt in range(KT):
                            nc.tensor.matmul(
                                qt_ps,
                                lhsT=W[:, kt, p * P:(p + 1) * P],
                                rhs=xt[:, kt, ch * 512:(ch + 1) * 512],
                                start=(kt == 0), stop=(kt == KT - 1),
                            )
                        qts = qtsp.tile([P, 512], BF16, tag="qts")
                        nc.vector.tensor_copy(qts, qt_ps)
                        out_ps = ps_out.tile([P, 512], F32, tag="psout")
                        nc.tensor.matmul(out_ps, lhsT=ctx_bd[:, p, :], rhs=qts,
                                         start=True, stop=True)
                        nc.vector.tensor_add(
                            xp_out[:, p, ch * 512:(ch + 1) * 512],
                            out_ps, xt[:, p, ch * 512:(ch + 1) * 512],
                        )

            def cross_out(o_r, ctx_bd, qpt):
                """o = merge(q @ ctx) + q_stream_residual, written transposed fp32 to DRAM."""
                for p in range(PAIRS):
                    for ch in range(CH):
                        out_ps = ps_out.tile([P, 512], F32, tag="psout")
                        nc.tensor.matmul(out_ps, lhsT=ctx_bd[:, p, :],
                                         rhs=qpt[:, p, ch * 512:(ch + 1) * 512],
                                         start=True, stop=True)
                        stg = outst.tile([P, 512], F32, tag="stg")
                        nc.vector.tensor_add(stg, out_ps,
                                             qpt[:, p, ch * 512:(ch + 1) * 512])
                        nc.sync.dma_start(
                            out=o_r[:, p, ch * 512:(ch + 1) * 512], in_=stg)

            # ---- self stage, stream 1 ----
            x1t = xbig.tile([P, KT, N], BF16, tag="xbig")
            nc.sync.dma_start(out=x1t, in_=x1T_r)
            Wsq = wts.tile([P, KT, 3 * C], BF16, tag="wts")
            nc.sync.dma_start(out=Wsq, in_=Wsq_r)
            x1p = xbig.tile([P, KT, N], BF16, tag="xbig")
            self_stage(x1t, Wsq, x1p)
            nc.sync.dma_start(out=x1p_r, in_=x1p)     # spill for later reload

            # ---- self stage, stream 2 ----
            x2t = xbig.tile([P, KT, N], BF16, tag="xbig")   # reuses x1t slot
            nc.sync.dma_start(out=x2t, in_=x2T_r)
            x2p = xbig.tile([P, KT, N], BF16, tag="xbig")   # reuses x1p slot
            self_stage(x2t, Wsq, x2p)

            # ---- cross stage ----
            Wkv2 = wts.tile([P, KT, 2 * C], BF16, tag="wts")
            nc.sync.dma_start(out=Wkv2, in_=Wkv2_r)
            ctx2_ps = ctx_accumulate(x2p, Wkv2, kvcol0=0)
            ctx2_bd = ctxsb.tile([P, PAIRS, P], BF16, tag="ctx_bd")
            for p in range(PAIRS):
                softmax_pair(ctx2_ps, p, ctx2_bd)

            x1pr = xbig.tile([P, KT, N], BF16, tag="xbig")  # reuses x2t slot
            nc.sync.dma_start(out=x1pr, in_=x1p_r)
            cross_out(o1T_r, ctx2_bd, x1pr)                 # o1 = q1 @ ctx2 + x1'

            Wkv1 = wts.tile([P, KT, 2 * C], BF16, tag="wts")
            nc.sync.dma_start(out=Wkv1, in_=Wkv1_r)
            ctx1_ps = ctx_accumulate(x1pr, Wkv1, kvcol0=0)
            ctx1_bd = ctxsb.tile([P, PAIRS, P], BF16, tag="ctx_bd")
            for p in range(PAIRS):
                softmax_pair(ctx1_ps, p, ctx1_bd)
            cross_out(o2T_r, ctx1_bd, x2p)                  # o2 = q2 @ ctx1 + x2'

    nc.finalize()
    return nc


def _get_nc():
    if "nc" not in _CACHE:
        _CACHE["nc"] = _build()
    return _CACHE["nc"]


def kernel(x1, x2, Wsqkv1, Wkv1, Wkv2, num_heads=16, selfattn=1, **_unused):
    x1 = np.asarray(x1, dtype=np.float32)
    x2 = np.asarray(x2, dtype=np.float32)
    Wsq_b = np.ascontiguousarray(np.asarray(Wsqkv1, np.float32)).astype(ml_dtypes.bfloat16)
    Wkv1_b = np.ascontiguousarray(np.asarray(Wkv1, np.float32)).astype(ml_dtypes.bfloat16)
    Wkv2_b = np.ascontiguousarray(np.asarray(Wkv2, np.float32)).astype(ml_dtypes.bfloat16)

    nc = _get_nc()
    in_maps = []
    for b in range(B):
        in_maps.append({
            "x1T": np.ascontiguousarray(x1[b].T).astype(ml_dtypes.bfloat16),
            "x2T": np.ascontiguousarray(x2[b].T).astype(ml_dtypes.bfloat16),
            "Wsqkv": Wsq_b,
            "Wkv1": Wkv1_b,
            "Wkv2": Wkv2_b,
        })
    res = run_bass_kernel_spmd(nc, in_maps, core_ids=list(range(B)),
                               trace=bool(int(os.environ.get("KERNEL_TRACE", "0"))))
    _CACHE["last_result"] = res
    o1 = np.stack([np.asarray(res.results[b]["o1T"], np.float32).T for b in range(B)])
    o2 = np.stack([np.asarray(res.results[b]["o2T"], np.float32).T for b in range(B)])
    return o1, o2



# revision 9
# speedup vs baseline: 15.8057x; 15.8057x over previous
"""Trainium2 Bass kernel for nn_CrossAttn (linear cross-attention, B=8 N=4096 C=1024 H=16).

Strategy:
  - Data-parallel over B across the 8 NeuronCores (batch-local math, no collectives).
  - Host pre-transposes activations to x^T [C, N] (C on partitions) and casts matmul
    operands to bf16; PSUM accumulation is fp32.
  - Self stage per stream: qkv GEMMs from x^T; linear ctx = softmax_d(k^T v * scale)
    accumulated as paired v^T k matmuls (2 heads -> one 128x128 MM); softmax along the
    free axis; ctx transposed via PE into a block-diagonal 2-head tile so the output
    product q @ ctx becomes (ctx_bd)^T @ q^T with K=128, N=512.
  - Cross stage: q is the post-self activation itself (already resident transposed);
    kv GEMMs with Wkv1/Wkv2; o1 = ctx2-product + x1', o2 = ctx1-product + x2'.
  - Outputs written transposed [C, N] fp32; host un-transposes.
"""

import os
import sys

sys.path.insert(0, "/opt/trn_rl_repo")

import numpy as np
import ml_dtypes

import concourse.bass as bass
import concourse.mybir as mybir
import concourse.tile as tile
from concourse import bacc
from concourse.masks import make_identity
from concourse.bass_utils import run_bass_kernel_spmd

B, N, C, H = 8, 4096, 1024, 16
D = C // H                 # 64
SCALE = D ** -0.5          # 0.125
P = 128                    # partitions
KT = C // P                # 8 contraction tiles
NT = N // P                # 32 n-tiles (ctx accumulation)
CH = N // 512              # 8 n-chunks of 512
PAIRS = H // 2             # 8 head pairs
F32 = mybir.dt.float32
BF16 = mybir.dt.bfloat16

_CACHE = {}


import contextlib


@contextlib.contextmanager
def _nullctx():
    yield


def _build(loop=True):
    nc = bacc.Bacc(None, target_bir_lowering=False)

    niter_d = nc.dram_tensor("niter", [1, 1], mybir.dt.int32, kind="ExternalInput")
    x1T_d = nc.dram_tensor("x1T", [C, N], BF16, kind="ExternalInput")
    x2T_d = nc.dram_tensor("x2T", [C, N], BF16, kind="ExternalInput")
    Wsqkv_d = nc.dram_tensor("Wsqkv", [C, 3 * C], BF16, kind="ExternalInput")
    Wkv1_d = nc.dram_tensor("Wkv1", [C, 2 * C], BF16, kind="ExternalInput")
    Wkv2_d = nc.dram_tensor("Wkv2", [C, 2 * C], BF16, kind="ExternalInput")
    o1T_d = nc.dram_tensor("o1T", [C, N], F32, kind="ExternalOutput")
    o2T_d = nc.dram_tensor("o2T", [C, N], F32, kind="ExternalOutput")
    x1p_scr = nc.dram_tensor("x1p_scratch", [C, N], BF16, kind="Internal")

    # (kt*128 + p, n) -> [p, kt, n] view for per-partition-tile DMA
    x1T_r = x1T_d[:].rearrange("(t p) n -> p t n", p=P)
    x2T_r = x2T_d[:].rearrange("(t p) n -> p t n", p=P)
    Wsq_r = Wsqkv_d[:].rearrange("(t p) c -> p t c", p=P)
    Wkv1_r = Wkv1_d[:].rearrange("(t p) c -> p t c", p=P)
    Wkv2_r = Wkv2_d[:].rearrange("(t p) c -> p t c", p=P)
    o1T_r = o1T_d[:].rearrange("(t p) n -> p t n", p=P)
    o2T_r = o2T_d[:].rearrange("(t p) n -> p t n", p=P)
    x1p_r = x1p_scr[:].rearrange("(t p) n -> p t n", p=P)

    with tile.TileContext(nc) as tc:
        with (
            tc.tile_pool(name="xbig", bufs=2) as xbig,
            tc.tile_pool(name="wts", bufs=1) as wts,
            tc.tile_pool(name="kvsb", bufs=2) as kvsb,
            tc.tile_pool(name="qts", bufs=3) as qtsp,
            tc.tile_pool(name="ctxsb", bufs=2) as ctxsb,
            tc.tile_pool(name="ctxacc", bufs=1) as ctxaccp,
            tc.tile_pool(name="smax", bufs=2) as smaxp,
            tc.tile_pool(name="stats", bufs=4) as stats,
            tc.tile_pool(name="outst", bufs=2) as outst,
            tc.tile_pool(name="singles", bufs=1) as singles,
            tc.tile_pool(name="ps_kv", bufs=2, space="PSUM") as ps_kv,
            tc.tile_pool(name="ps_ctx", bufs=1, space="PSUM") as ps_ctx,
            tc.tile_pool(name="ps_qt", bufs=2, space="PSUM") as ps_qt,
            tc.tile_pool(name="ps_out", bufs=2, space="PSUM") as ps_out,
        ):
            nit_sb = singles.tile([1, 1], mybir.dt.int32, tag="nit")
            nc.sync.dma_start(out=nit_sb, in_=niter_d[:])
            niter_v = nc.values_load(nit_sb[0:1, 0:1], min_val=1, max_val=256,
                                     skip_runtime_bounds_check=True)

            ident = singles.tile([P, P], F32)
            make_identity(nc, ident)

            def ctx_accumulate(xt, W, kvcol0):
                """Accumulate per-pair ctx_rawT = v^T k over all n-tiles.

                xt: [P, KT, N] bf16 (activation transposed), W: [P, KT, wcols] bf16.
                Returns SBUF tile [P, PAIRS*128] fp32: pair p cols [128p,128p+128),
                head 2p block at rows 0:64 cols +0:64, head 2p+1 at rows 64:128
                cols +64:128 (off-diagonal blocks are garbage, never read).

                Accumulation across n-tiles happens in SBUF via DVE adds — four
                per-pair PSUM groups would share a bank, and each group's
                start=True clears has_written for the WHOLE bank, corrupting the
                other pairs' accumulation.
                """
                ctx_acc = ctxaccp.tile([P, PAIRS * P], F32, tag="ctxacc")

                def pair_mms(kv, nt):
                    ctx_ps = ps_ctx.tile([P, PAIRS * P], F32, tag="ctx")
                    for p in range(PAIRS):
                        nc.tensor.matmul(
                            ctx_ps[:, p * P:(p + 1) * P],
                            lhsT=kv[:, C + p * P: C + (p + 1) * P],   # v pair
                            rhs=kv[:, p * P:(p + 1) * P],             # k pair
                            start=True, stop=True,
                        )
                    if nt == 0:
                        nc.vector.tensor_copy(ctx_acc, ctx_ps)
                    else:
                        nc.vector.tensor_add(ctx_acc, ctx_acc, ctx_ps)

                prev = None
                for nt in range(NT):
                    kv = kvsb.tile([P, 2 * C], BF16, tag="kv")
                    for ch in range(4):
                        kv_ps = ps_kv.tile([P, 512], F32, tag="kvps")
                        for kt in range(KT):
                            nc.tensor.matmul(
                                kv_ps,
                                lhsT=xt[:, kt, nt * P:(nt + 1) * P],
                                rhs=W[:, kt, kvcol0 + ch * 512: kvcol0 + (ch + 1) * 512],
                                start=(kt == 0), stop=(kt == KT - 1),
                            )
                        nc.vector.tensor_copy(kv[:, ch * 512:(ch + 1) * 512], kv_ps)
                    if prev is not None:
                        pair_mms(*prev)
                    prev = (kv, nt)
                pair_mms(*prev)
                return ctx_acc

            def softmax_pair(ctx_ps, p, ctx_bd):
                """Softmax over d (free axis) of the two diag blocks of pair p, then
                PE-transpose into slice p of the block-diagonal bf16 ctx tile."""
                S = smaxp.tile([P, P], F32, tag="smax")
                nc.vector.memset(S, 0.0)
                for r0 in (0, 64):
                    blk = ctx_ps[r0:r0 + 64, p * P + r0: p * P + r0 + 64]
                    mx = stats.tile([P, 1], F32, tag="mx")
                    nc.vector.reduce_max(mx[r0:r0 + 64], blk, axis=mybir.AxisListType.X)
                    ng = stats.tile([P, 1], F32, tag="ng")
                    nc.scalar.mul(ng[r0:r0 + 64], mx[r0:r0 + 64], -SCALE)
                    se = stats.tile([P, 1], F32, tag="se")
                    nc.scalar.activation(
                        S[r0:r0 + 64, r0:r0 + 64], blk,
                        mybir.ActivationFunctionType.Exp,
                        bias=ng[r0:r0 + 64], scale=SCALE,
                        accum_out=se[r0:r0 + 64],
                    )
                    rv = stats.tile([P, 1], F32, tag="rv")
                    nc.vector.reciprocal(rv[r0:r0 + 64], se[r0:r0 + 64])
                    nc.vector.tensor_scalar_mul(
                        S[r0:r0 + 64, r0:r0 + 64], S[r0:r0 + 64, r0:r0 + 64],
                        rv[r0:r0 + 64],
                    )
                tr_ps = ps_out.tile([P, P], F32, tag="psout")
                nc.tensor.transpose(tr_ps, S, ident)
                nc.vector.tensor_copy(ctx_bd[:, p, :], tr_ps)

            def self_stage(xt, W, xp_out):
                """One self-attention branch: returns nothing; writes x' (bf16,
                transposed) into xp_out [P, KT, N]."""
                ctx_ps = ctx_accumulate(xt, W, kvcol0=C)
                ctx_bd = ctxsb.tile([P, PAIRS, P], BF16, tag="ctx_bd")
                for p in range(PAIRS):
                    softmax_pair(ctx_ps, p, ctx_bd)
                    for ch in range(CH):
                        qt_ps = ps_qt.tile([P, 512], F32, tag="qt")
                        for kt in range(KT):
                            nc.tensor.matmul(
                                qt_ps,
                                lhsT=W[:, kt, p * P:(p + 1) * P],
                                rhs=xt[:, kt, ch * 512:(ch + 1) * 512],
                                start=(kt == 0), stop=(kt == KT - 1),
                            )
                        qts = qtsp.tile([P, 512], BF16, tag="qts")
                        nc.vector.tensor_copy(qts, qt_ps)
                        out_ps = ps_out.tile([P, 512], F32, tag="psout")
                        nc.tensor.matmul(out_ps, lhsT=ctx_bd[:, p, :], rhs=qts,
                                         start=True, stop=True)
                        nc.vector.tensor_add(
                            xp_out[:, p, ch * 512:(ch + 1) * 512],
                            out_ps, xt[:, p, ch * 512:(ch + 1) * 512],
                        )

            def cross_out(o_r, ctx_bd, qpt):
                """o = merge(q @ ctx) + q_stream_residual, written transposed fp32 to DRAM."""
                for p in range(PAIRS):
                    for ch in range(CH):
                        out_ps = ps_out.tile([P, 512], F32, tag="psout")
                        nc.tensor.matmul(out_ps, lhsT=ctx_bd[:, p, :],
                                         rhs=qpt[:, p, ch * 512:(ch + 1) * 512],
                                         start=True, stop=True)
                        stg = outst.tile([P, 512], F32, tag="stg")
                        nc.vector.tensor_add(stg, out_ps,
                                             qpt[:, p, ch * 512:(ch + 1) * 512])
                        nc.sync.dma_start(
                            out=o_r[:, p, ch * 512:(ch + 1) * 512], in_=stg)

            with (tc.For_i(0, niter_v, 1, name="rep") if loop else _nullctx()):
                # ---- self stage, stream 1 ----
                x1t = xbig.tile([P, KT, N], BF16, tag="xbig")
                nc.sync.dma_start(out=x1t, in_=x1T_r)
                Wsq = wts.tile([P, KT, 3 * C], BF16, tag="wts")
                nc.sync.dma_start(out=Wsq, in_=Wsq_r)
                x1p = xbig.tile([P, KT, N], BF16, tag="xbig")
                self_stage(x1t, Wsq, x1p)
                nc.sync.dma_start(out=x1p_r, in_=x1p)     # spill for later reload

                # ---- self stage, stream 2 ----
                x2t = xbig.tile([P, KT, N], BF16, tag="xbig")   # reuses x1t slot
                nc.sync.dma_start(out=x2t, in_=x2T_r)
                x2p = xbig.tile([P, KT, N], BF16, tag="xbig")   # reuses x1p slot
                self_stage(x2t, Wsq, x2p)

                # ---- cross stage ----
                Wkv2 = wts.tile([P, KT, 2 * C], BF16, tag="wts")
                nc.sync.dma_start(out=Wkv2, in_=Wkv2_r)
                ctx2_ps = ctx_accumulate(x2p, Wkv2, kvcol0=0)
                ctx2_bd = ctxsb.tile([P, PAIRS, P], BF16, tag="ctx_bd")
                for p in range(PAIRS):
                    softmax_pair(ctx2_ps, p, ctx2_bd)

                x1pr = xbig.tile([P, KT, N], BF16, tag="xbig")  # reuses x2t slot
                nc.sync.dma_start(out=x1pr, in_=x1p_r)
                cross_out(o1T_r, ctx2_bd, x1pr)                 # o1 = q1 @ ctx2 + x1'

                Wkv1 = wts.tile([P, KT, 2 * C], BF16, tag="wts")
                nc.sync.dma_start(out=Wkv1, in_=Wkv1_r)
                ctx1_ps = ctx_accumulate(x1pr, Wkv1, kvcol0=0)
                ctx1_bd = ctxsb.tile([P, PAIRS, P], BF16, tag="ctx_bd")
                for p in range(PAIRS):
                    softmax_pair(ctx1_ps, p, ctx1_bd)
                cross_out(o2T_r, ctx1_bd, x2p)                  # o2 = q2 @ ctx1 + x2'

    nc.finalize()
    return nc


def _get_nc():
    if "nc" not in _CACHE:
        _CACHE["nc"] = _build()
    return _CACHE["nc"]


def make_in_maps(x1, x2, Wsqkv1, Wkv1, Wkv2, niter=1):
    x1 = np.asarray(x1, dtype=np.float32)
    x2 = np.asarray(x2, dtype=np.float32)
    Wsq_b = np.ascontiguousarray(np.asarray(Wsqkv1, np.float32)).astype(ml_dtypes.bfloat16)
    Wkv1_b = np.ascontiguousarray(np.asarray(Wkv1, np.float32)).astype(ml_dtypes.bfloat16)
    Wkv2_b = np.ascontiguousarray(np.asarray(Wkv2, np.float32)).astype(ml_dtypes.bfloat16)
    nit = np.array([[niter]], dtype=np.int32)
    in_maps = []
    for b in range(B):
        in_maps.append({
            "niter": nit,
            "x1T": np.ascontiguousarray(x1[b].T).astype(ml_dtypes.bfloat16),
            "x2T": np.ascontiguousarray(x2[b].T).astype(ml_dtypes.bfloat16),
            "Wsqkv": Wsq_b,
            "Wkv1": Wkv1_b,
            "Wkv2": Wkv2_b,
        })
    return in_maps


def gather_outputs(results):
    o1 = np.stack([np.asarray(results[b]["o1T"], np.float32).T for b in range(B)])
    o2 = np.stack([np.asarray(results[b]["o2T"], np.float32).T for b in range(B)])
    return o1, o2


def kernel(x1, x2, Wsqkv1, Wkv1, Wkv2, num_heads=16, selfattn=1, **_unused):
    in_maps = make_in_maps(x1, x2, Wsqkv1, Wkv1, Wkv2)
    nc = _get_nc()
    res = run_bass_kernel_spmd(nc, in_maps, core_ids=list(range(B)),
                               trace=bool(int(os.environ.get("KERNEL_TRACE", "0"))))
    _CACHE["last_result"] = res
    return gather_outputs(res.results)



# revision 10
# speedup vs baseline: 42.8096x; 2.7085x over previous
"""Trainium2 Bass kernel for nn_CrossAttn (linear cross-attention, B=8 N=4096 C=1024 H=16).

v3 changes over v2 (engine-rebalanced baseline):
  - bf16 DRAM outputs (host converts to fp32): halves output DMA bytes and
    staging size; outst bufs=6 so several output DMAs stay in flight and the
    per-(pair,chunk) matmul->add->dma pipeline is no longer DMA-receipt bound.
  - x1' is never SBUF-resident: self stage 1 streams it to DRAM through the
    staging pool, freeing an SBUF slot so x2 prefetches during self stage 1
    (removes a ~50us PE stall at the stream-1 -> stream-2 transition).
  - Inputs/weights DMA in functional-order chunks (kv weight columns before q
    columns, x in 512-token slices) so the first kv GEMMs start ~3us in.
  - PSUM->SBUF evacuations on ScalarE; DVE keeps residual adds + ctx accum.
  - On-device repeat loop (niter input) for RTT-free timing.
"""

import os
import sys

sys.path.insert(0, "/opt/trn_rl_repo")

import numpy as np
import ml_dtypes

import concourse.bass as bass
import concourse.mybir as mybir
import concourse.tile as tile
from concourse import bacc
from concourse.masks import make_identity
from concourse.bass_utils import run_bass_kernel_spmd

B, N, C, H = 8, 4096, 1024, 16
D = C // H                 # 64
SCALE = D ** -0.5          # 0.125
P = 128                    # partitions
KT = C // P                # 8 contraction tiles
NT = N // P                # 32 n-tiles (ctx accumulation)
CH = N // 512              # 8 n-chunks of 512
PAIRS = H // 2             # 8 head pairs
F32 = mybir.dt.float32
BF16 = mybir.dt.bfloat16

_CACHE = {}


import contextlib


@contextlib.contextmanager
def _nullctx():
    yield


def _build(loop=True):
    nc = bacc.Bacc(None, target_bir_lowering=False)

    niter_d = nc.dram_tensor("niter", [1, 1], mybir.dt.int32, kind="ExternalInput")
    x1T_d = nc.dram_tensor("x1T", [C, N], BF16, kind="ExternalInput")
    x2T_d = nc.dram_tensor("x2T", [C, N], BF16, kind="ExternalInput")
    Wsqkv_d = nc.dram_tensor("Wsqkv", [C, 3 * C], BF16, kind="ExternalInput")
    Wkv1_d = nc.dram_tensor("Wkv1", [C, 2 * C], BF16, kind="ExternalInput")
    Wkv2_d = nc.dram_tensor("Wkv2", [C, 2 * C], BF16, kind="ExternalInput")
    o1T_d = nc.dram_tensor("o1T", [C, N], BF16, kind="ExternalOutput")
    o2T_d = nc.dram_tensor("o2T", [C, N], BF16, kind="ExternalOutput")
    x1p_scr = nc.dram_tensor("x1p_scratch", [C, N], BF16, kind="Internal")

    # (kt*128 + p, n) -> [p, kt, n] view for per-partition-tile DMA
    x1T_r = x1T_d[:].rearrange("(t p) n -> p t n", p=P)
    x2T_r = x2T_d[:].rearrange("(t p) n -> p t n", p=P)
    Wsq_r = Wsqkv_d[:].rearrange("(t p) c -> p t c", p=P)
    Wkv1_r = Wkv1_d[:].rearrange("(t p) c -> p t c", p=P)
    Wkv2_r = Wkv2_d[:].rearrange("(t p) c -> p t c", p=P)
    o1T_r = o1T_d[:].rearrange("(t p) n -> p t n", p=P)
    o2T_r = o2T_d[:].rearrange("(t p) n -> p t n", p=P)
    x1p_r = x1p_scr[:].rearrange("(t p) n -> p t n", p=P)

    with tile.TileContext(nc) as tc:
        with (
            tc.tile_pool(name="xbig", bufs=2) as xbig,
            tc.tile_pool(name="wts", bufs=1) as wts,
            tc.tile_pool(name="kvsb", bufs=2) as kvsb,
            tc.tile_pool(name="qts", bufs=3) as qtsp,
            tc.tile_pool(name="ctxsb", bufs=2) as ctxsb,
            tc.tile_pool(name="ctxacc", bufs=1) as ctxaccp,
            tc.tile_pool(name="smax", bufs=2) as smaxp,
            tc.tile_pool(name="stats", bufs=4) as stats,
            tc.tile_pool(name="outst", bufs=6) as outst,
            tc.tile_pool(name="singles", bufs=1) as singles,
            tc.tile_pool(name="ps_kv", bufs=2, space="PSUM") as ps_kv,
            tc.tile_pool(name="ps_ctx", bufs=1, space="PSUM") as ps_ctx,
            tc.tile_pool(name="ps_qt", bufs=2, space="PSUM") as ps_qt,
            tc.tile_pool(name="ps_out", bufs=2, space="PSUM") as ps_out,
        ):
            nit_sb = singles.tile([1, 1], mybir.dt.int32, tag="nit")
            nc.sync.dma_start(out=nit_sb, in_=niter_d[:])
            niter_v = nc.values_load(nit_sb[0:1, 0:1], min_val=1, max_val=256,
                                     skip_runtime_bounds_check=True)

            ident = singles.tile([P, P], F32)
            make_identity(nc, ident)

            def load_x(dst, src_r):
                for ch in range(CH):
                    nc.sync.dma_start(out=dst[:, :, ch * 512:(ch + 1) * 512],
                                      in_=src_r[:, :, ch * 512:(ch + 1) * 512])

            def load_w(dst, src_r, col_order):
                for c0 in col_order:
                    nc.sync.dma_start(out=dst[:, :, c0:c0 + 512],
                                      in_=src_r[:, :, c0:c0 + 512])

            def ctx_accumulate(xt, W, kvcol0):
                """Accumulate per-pair ctx_rawT = v^T k over all n-tiles.

                Returns SBUF tile [P, PAIRS*128] fp32: pair p cols [128p,128p+128),
                head 2p diag block rows 0:64, head 2p+1 rows 64:128 (off-diagonal
                garbage never read). SBUF accumulation via DVE adds (per-pair PSUM
                groups would share a bank and corrupt each other's has_written)."""
                ctx_acc = ctxaccp.tile([P, PAIRS * P], F32, tag="ctxacc")

                def pair_mms(kv, nt):
                    ctx_ps = ps_ctx.tile([P, PAIRS * P], F32, tag="ctx")
                    for p in range(PAIRS):
                        nc.tensor.matmul(
                            ctx_ps[:, p * P:(p + 1) * P],
                            lhsT=kv[:, C + p * P: C + (p + 1) * P],   # v pair
                            rhs=kv[:, p * P:(p + 1) * P],             # k pair
                            start=True, stop=True,
                        )
                    if nt == 0:
                        nc.vector.tensor_copy(ctx_acc, ctx_ps)
                    else:
                        nc.vector.tensor_add(ctx_acc, ctx_acc, ctx_ps)

                prev = None
                for nt in range(NT):
                    kv = kvsb.tile([P, 2 * C], BF16, tag="kv")
                    for ch in range(4):
                        kv_ps = ps_kv.tile([P, 512], F32, tag="kvps")
                        for kt in range(KT):
                            nc.tensor.matmul(
                                kv_ps,
                                lhsT=xt[:, kt, nt * P:(nt + 1) * P],
                                rhs=W[:, kt, kvcol0 + ch * 512: kvcol0 + (ch + 1) * 512],
                                start=(kt == 0), stop=(kt == KT - 1),
                            )
                        nc.scalar.copy(kv[:, ch * 512:(ch + 1) * 512], kv_ps)
                    if prev is not None:
                        pair_mms(*prev)
                    prev = (kv, nt)
                pair_mms(*prev)
                return ctx_acc

            def softmax_pair(ctx_sb, p, ctx_bd):
                """Softmax over d (free axis) of the two diag blocks of pair p, then
                PE-transpose into slice p of the block-diagonal bf16 ctx tile."""
                S = smaxp.tile([P, P], F32, tag="smax")
                nc.vector.memset(S, 0.0)
                for r0 in (0, 64):
                    blk = ctx_sb[r0:r0 + 64, p * P + r0: p * P + r0 + 64]
                    mx = stats.tile([P, 1], F32, tag="mx")
                    nc.vector.reduce_max(mx[r0:r0 + 64], blk, axis=mybir.AxisListType.X)
                    ng = stats.tile([P, 1], F32, tag="ng")
                    nc.scalar.mul(ng[r0:r0 + 64], mx[r0:r0 + 64], -SCALE)
                    se = stats.tile([P, 1], F32, tag="se")
                    nc.scalar.activation(
                        S[r0:r0 + 64, r0:r0 + 64], blk,
                        mybir.ActivationFunctionType.Exp,
                        bias=ng[r0:r0 + 64], scale=SCALE,
                        accum_out=se[r0:r0 + 64],
                    )
                    rv = stats.tile([P, 1], F32, tag="rv")
                    nc.vector.reciprocal(rv[r0:r0 + 64], se[r0:r0 + 64])
                    nc.vector.tensor_scalar_mul(
                        S[r0:r0 + 64, r0:r0 + 64], S[r0:r0 + 64, r0:r0 + 64],
                        rv[r0:r0 + 64],
                    )
                tr_ps = ps_out.tile([P, P], F32, tag="psout")
                nc.tensor.transpose(tr_ps, S, ident)
                nc.vector.tensor_copy(ctx_bd[:, p, :], tr_ps)

            def self_stage(xt, W, xp_out=None, spill_r=None):
                """One self-attention branch. x' (bf16, transposed) goes to the
                SBUF tile xp_out, or streams to DRAM via spill_r."""
                ctx_sb = ctx_accumulate(xt, W, kvcol0=C)
                ctx_bd = ctxsb.tile([P, PAIRS, P], BF16, tag="ctx_bd")
                for p in range(PAIRS):
                    softmax_pair(ctx_sb, p, ctx_bd)
                    for ch in range(CH):
                        qt_ps = ps_qt.tile([P, 512], F32, tag="qt")
                        for kt in range(KT):
                            nc.tensor.matmul(
                                qt_ps,
                                lhsT=W[:, kt, p * P:(p + 1) * P],
                                rhs=xt[:, kt, ch * 512:(ch + 1) * 512],
                                start=(kt == 0), stop=(kt == KT - 1),
                            )
                        qts = qtsp.tile([P, 512], BF16, tag="qts")
                        nc.scalar.copy(qts, qt_ps)
                        out_ps = ps_out.tile([P, 512], F32, tag="psout")
                        nc.tensor.matmul(out_ps, lhsT=ctx_bd[:, p, :], rhs=qts,
                                         start=True, stop=True)
                        if xp_out is not None:
                            nc.vector.tensor_add(
                                xp_out[:, p, ch * 512:(ch + 1) * 512],
                                out_ps, xt[:, p, ch * 512:(ch + 1) * 512],
                            )
                        else:
                            stg = outst.tile([P, 512], BF16, tag="stg")
                            nc.vector.tensor_add(
                                stg, out_ps, xt[:, p, ch * 512:(ch + 1) * 512])
                            nc.sync.dma_start(
                                out=spill_r[:, p, ch * 512:(ch + 1) * 512], in_=stg)

            def cross_out(o_r, ctx_bd, qpt):
                """o = merge(q @ ctx) + q_stream_residual, written transposed bf16."""
                for p in range(PAIRS):
                    for ch in range(CH):
                        out_ps = ps_out.tile([P, 512], F32, tag="psout")
                        nc.tensor.matmul(out_ps, lhsT=ctx_bd[:, p, :],
                                         rhs=qpt[:, p, ch * 512:(ch + 1) * 512],
                                         start=True, stop=True)
                        stg = outst.tile([P, 512], BF16, tag="stg")
                        nc.vector.tensor_add(stg, out_ps,
                                             qpt[:, p, ch * 512:(ch + 1) * 512])
                        nc.sync.dma_start(
                            out=o_r[:, p, ch * 512:(ch + 1) * 512], in_=stg)

            with (tc.For_i(0, niter_v, 1, name="rep") if loop else _nullctx()):
                # ---- self stage, stream 1 (x1' streamed to DRAM) ----
                # DMA in consumption order: the HWDGE ring is FIFO, so the
                # first kv GEMM (x chunk 0 + kv weight cols) must queue first.
                x1t = xbig.tile([P, KT, N], BF16, tag="xbig")
                nc.sync.dma_start(out=x1t[:, :, 0:512], in_=x1T_r[:, :, 0:512])
                Wsq = wts.tile([P, KT, 3 * C], BF16, tag="wts")
                load_w(Wsq, Wsq_r, [C, C + 512, 2 * C, 2 * C + 512])
                for ch in range(1, CH):
                    nc.sync.dma_start(out=x1t[:, :, ch * 512:(ch + 1) * 512],
                                      in_=x1T_r[:, :, ch * 512:(ch + 1) * 512])
                load_w(Wsq, Wsq_r, [0, 512])
                self_stage(x1t, Wsq, spill_r=x1p_r)

                # ---- self stage, stream 2 (x2 prefetched during stage 1) ----
                x2t = xbig.tile([P, KT, N], BF16, tag="xbig")
                load_x(x2t, x2T_r)
                x2p = xbig.tile([P, KT, N], BF16, tag="xbig")   # reuses x1t slot
                self_stage(x2t, Wsq, xp_out=x2p)

                # ---- cross stage ----
                Wkv2 = wts.tile([P, KT, 2 * C], BF16, tag="wts")
                load_w(Wkv2, Wkv2_r, [0, 512, 1024, 1536])
                ctx2_sb = ctx_accumulate(x2p, Wkv2, kvcol0=0)
                ctx2_bd = ctxsb.tile([P, PAIRS, P], BF16, tag="ctx_bd")
                for p in range(PAIRS):
                    softmax_pair(ctx2_sb, p, ctx2_bd)

                x1pr = xbig.tile([P, KT, N], BF16, tag="xbig")  # reuses x2t slot
                load_x(x1pr, x1p_r)
                cross_out(o1T_r, ctx2_bd, x1pr)                 # o1 = q1 @ ctx2 + x1'

                Wkv1 = wts.tile([P, KT, 2 * C], BF16, tag="wts")
                load_w(Wkv1, Wkv1_r, [0, 512, 1024, 1536])
                ctx1_sb = ctx_accumulate(x1pr, Wkv1, kvcol0=0)
                ctx1_bd = ctxsb.tile([P, PAIRS, P], BF16, tag="ctx_bd")
                for p in range(PAIRS):
                    softmax_pair(ctx1_sb, p, ctx1_bd)
                cross_out(o2T_r, ctx1_bd, x2p)                  # o2 = q2 @ ctx1 + x2'

    nc.finalize()
    return nc


def _get_nc():
    if "nc" not in _CACHE:
        _CACHE["nc"] = _build()
    return _CACHE["nc"]


def make_in_maps(x1, x2, Wsqkv1, Wkv1, Wkv2, niter=1):
    x1 = np.asarray(x1, dtype=np.float32)
    x2 = np.asarray(x2, dtype=np.float32)
    Wsq_b = np.ascontiguousarray(np.asarray(Wsqkv1, np.float32)).astype(ml_dtypes.bfloat16)
    Wkv1_b = np.ascontiguousarray(np.asarray(Wkv1, np.float32)).astype(ml_dtypes.bfloat16)
    Wkv2_b = np.ascontiguousarray(np.asarray(Wkv2, np.float32)).astype(ml_dtypes.bfloat16)
    nit = np.array([[niter]], dtype=np.int32)
    in_maps = []
    for b in range(B):
        in_maps.append({
            "niter": nit,
            "x1T": np.ascontiguousarray(x1[b].T).astype(ml_dtypes.bfloat16),
            "x2T": np.ascontiguousarray(x2[b].T).astype(ml_dtypes.bfloat16),
            "Wsqkv": Wsq_b,
            "Wkv1": Wkv1_b,
            "Wkv2": Wkv2_b,
        })
    return in_maps


def gather_outputs(results):
    o1 = np.stack([np.asarray(results[b]["o1T"]).astype(np.float32).T for b in range(B)])
    o2 = np.stack([np.asarray(results[b]["o2T"]).astype(np.float32).T for b in range(B)])
    return o1, o2


def kernel(x1, x2, Wsqkv1, Wkv1, Wkv2, num_heads=16, selfattn=1, **_unused):
    in_maps = make_in_maps(x1, x2, Wsqkv1, Wkv1, Wkv2)
    nc = _get_nc()
    res = run_bass_kernel_spmd(nc, in_maps, core_ids=list(range(B)),
                               trace=bool(int(os.environ.get("KERNEL_TRACE", "0"))))
    _CACHE["last_result"] = res
    return gather_outputs(res.results)


# revision 11
# speedup vs baseline: 422.7144x; 9.8743x over previous
"""Trainium2 Bass kernel for nn_CrossAttn (linear cross-attention, B=8 N=4096 C=1024 H=16).

v4 changes over v3:
  - Self-stage ctx via Gram matrix: k,v are never used outside ctx, and
    ctx_rawT = v^T k = Wv^T (x^T x) Wk. Computing G = x^T x (512 MMs), then
    Bv = G Wv (128 MMs), then per-pair Bv^T Wk (64 small MMs) replaces the
    2C-column kv projection (1024 MMs + 256 small MMs): ~25% less PE work
    per self stream. G comes from x in natural token-major layout (new x1N/
    x2N inputs, no host transpose).
  - Both post-self activations spill to DRAM scratch; second-pass consumers
    (self q/out phase, cross kv GEMMs, cross outputs) stream chunks through
    small SBUF pools, which frees the SBUF for G/Bv.
  - bf16 outputs, deep output staging, ScalarE PSUM evacuation, chunked
    consumption-ordered DMA, on-device repeat loop (niter) as in v3.
"""

import os
import sys

sys.path.insert(0, "/opt/trn_rl_repo")

import numpy as np
import ml_dtypes

import concourse.bass as bass
import concourse.mybir as mybir
import concourse.tile as tile
from concourse import bacc
from concourse.masks import make_identity
from concourse.bass_utils import run_bass_kernel_spmd

B, N, C, H = 8, 4096, 1024, 16
D = C // H                 # 64
SCALE = D ** -0.5          # 0.125
P = 128                    # partitions
KT = C // P                # 8 contraction tiles
NT = N // P                # 32 n-tiles
CH = N // 512              # 8 n-chunks of 512
PAIRS = H // 2             # 8 head pairs
F32 = mybir.dt.float32
BF16 = mybir.dt.bfloat16

_CACHE = {}


import contextlib


@contextlib.contextmanager
def _nullctx():
    yield


def _build(loop=True):
    nc = bacc.Bacc(None, target_bir_lowering=False)

    niter_d = nc.dram_tensor("niter", [1, 1], mybir.dt.int32, kind="ExternalInput")
    x1T_d = nc.dram_tensor("x1T", [C, N], BF16, kind="ExternalInput")
    x2T_d = nc.dram_tensor("x2T", [C, N], BF16, kind="ExternalInput")
    x1N_d = nc.dram_tensor("x1N", [N, C], BF16, kind="ExternalInput")
    x2N_d = nc.dram_tensor("x2N", [N, C], BF16, kind="ExternalInput")
    Wsqkv_d = nc.dram_tensor("Wsqkv", [C, 3 * C], BF16, kind="ExternalInput")
    Wkv1_d = nc.dram_tensor("Wkv1", [C, 2 * C], BF16, kind="ExternalInput")
    Wkv2_d = nc.dram_tensor("Wkv2", [C, 2 * C], BF16, kind="ExternalInput")
    o1T_d = nc.dram_tensor("o1T", [C, N], BF16, kind="ExternalOutput")
    o2T_d = nc.dram_tensor("o2T", [C, N], BF16, kind="ExternalOutput")
    x1p_scr = nc.dram_tensor("x1p_scratch", [C, N], BF16, kind="Internal")
    x2p_scr = nc.dram_tensor("x2p_scratch", [C, N], BF16, kind="Internal")

    x1T_r = x1T_d[:].rearrange("(t p) n -> p t n", p=P)
    x2T_r = x2T_d[:].rearrange("(t p) n -> p t n", p=P)
    x1N_r = x1N_d[:].rearrange("(t p) c -> p t c", p=P)
    x2N_r = x2N_d[:].rearrange("(t p) c -> p t c", p=P)
    Wsq_r = Wsqkv_d[:].rearrange("(t p) c -> p t c", p=P)
    Wkv1_r = Wkv1_d[:].rearrange("(t p) c -> p t c", p=P)
    Wkv2_r = Wkv2_d[:].rearrange("(t p) c -> p t c", p=P)
    o1T_r = o1T_d[:].rearrange("(t p) n -> p t n", p=P)
    o2T_r = o2T_d[:].rearrange("(t p) n -> p t n", p=P)
    x1p_r = x1p_scr[:].rearrange("(t p) n -> p t n", p=P)
    x2p_r = x2p_scr[:].rearrange("(t p) n -> p t n", p=P)

    with tile.TileContext(nc) as tc:
        with (
            tc.tile_pool(name="xn", bufs=1) as xnp,           # 64K: token-major x
            tc.tile_pool(name="gsb", bufs=1) as gsb,          # 16K: Gram matrix
            tc.tile_pool(name="btsb", bufs=1) as btsb,        # 16K: Bv = G Wv
            tc.tile_pool(name="xs", bufs=3) as xsp,           # 3x8K: streamed xT chunks
            tc.tile_pool(name="qrow", bufs=6) as qrowp,       # 3x1K: streamed pair rows
            tc.tile_pool(name="wts", bufs=1) as wts,          # 48K
            tc.tile_pool(name="kvsb", bufs=2) as kvsb,        # 2x4K
            tc.tile_pool(name="qts", bufs=3) as qtsp,
            tc.tile_pool(name="ctxsb", bufs=2) as ctxsb,
            tc.tile_pool(name="ctxacc", bufs=1) as ctxaccp,
            tc.tile_pool(name="smax", bufs=2) as smaxp,
            tc.tile_pool(name="stats", bufs=4) as stats,
            tc.tile_pool(name="outst", bufs=6) as outst,
            tc.tile_pool(name="singles", bufs=1) as singles,
            tc.tile_pool(name="ps_kv", bufs=2, space="PSUM") as ps_kv,
            tc.tile_pool(name="ps_ctx", bufs=1, space="PSUM") as ps_ctx,
            tc.tile_pool(name="ps_qt", bufs=2, space="PSUM") as ps_qt,
            tc.tile_pool(name="ps_out", bufs=2, space="PSUM") as ps_out,
        ):
            nit_sb = singles.tile([1, 1], mybir.dt.int32, tag="nit")
            nc.sync.dma_start(out=nit_sb, in_=niter_d[:])
            niter_v = nc.values_load(nit_sb[0:1, 0:1], min_val=1, max_val=256,
                                     skip_runtime_bounds_check=True)

            ident = singles.tile([P, P], F32)
            make_identity(nc, ident)

            def gram_ctx(xN, W):
                """Self-stream ctx_rawT via the Gram matrix.

                xN: [P, NT, C] bf16 token-major x. W: Wsqkv [P, KT, 3C]
                (k cols C..2C, v cols 2C..3C). Returns SBUF [P, PAIRS*128]
                fp32 with per-pair diag blocks of v^T k (same layout as the
                direct path)."""
                G = gsb.tile([P, KT, C], BF16, tag="G")
                for it in range(KT):
                    for jc in range(2):
                        g_ps = ps_kv.tile([P, 512], F32, tag="kvps")
                        for tn in range(NT):
                            nc.tensor.matmul(
                                g_ps,
                                lhsT=xN[:, tn, it * P:(it + 1) * P],
                                rhs=xN[:, tn, jc * 512:(jc + 1) * 512],
                                start=(tn == 0), stop=(tn == NT - 1),
                            )
                        nc.scalar.copy(G[:, it, jc * 512:(jc + 1) * 512], g_ps)
                BT = btsb.tile([P, KT, C], BF16, tag="BT")   # Bv[j, v] = (G @ Wv)[j, v]
                for jt in range(KT):
                    for vc in range(2):
                        b_ps = ps_kv.tile([P, 512], F32, tag="kvps")
                        for it in range(KT):
                            nc.tensor.matmul(
                                b_ps,
                                lhsT=G[:, it, jt * P:(jt + 1) * P],
                                rhs=W[:, it, 2 * C + vc * 512: 2 * C + (vc + 1) * 512],
                                start=(it == 0), stop=(it == KT - 1),
                            )
                        nc.scalar.copy(BT[:, jt, vc * 512:(vc + 1) * 512], b_ps)
                # ctx_rawT[e,d] = sum_j Bv[j,e] Wk[j,d], per-pair blocks
                ctx_acc = ctxaccp.tile([P, PAIRS * P], F32, tag="ctxacc")
                for jt in range(KT):
                    ctx_ps = ps_ctx.tile([P, PAIRS * P], F32, tag="ctx")
                    for p in range(PAIRS):
                        nc.tensor.matmul(
                            ctx_ps[:, p * P:(p + 1) * P],
                            lhsT=BT[:, jt, p * P:(p + 1) * P],
                            rhs=W[:, jt, C + p * P: C + (p + 1) * P],
                            start=True, stop=True,
                        )
                    if jt == 0:
                        nc.vector.tensor_copy(ctx_acc, ctx_ps)
                    else:
                        nc.vector.tensor_add(ctx_acc, ctx_acc, ctx_ps)
                return ctx_acc

            def ctx_accumulate_s(src_r, W, kvcol0, fuse=None):
                """Direct kv-projection ctx (cross stage), streaming x^T chunks
                from DRAM. Returns SBUF [P, PAIRS*128] fp32.

                fuse=(ctx_bd_other, o_r): while each chunk is resident, also
                emit the other stream's output product o = ctx_bd_other^T q + q
                from the same data (saves a full re-read of src_r)."""
                ctx_acc = ctxaccp.tile([P, PAIRS * P], F32, tag="ctxacc")

                def pair_mms(kv, nt):
                    ctx_ps = ps_ctx.tile([P, PAIRS * P], F32, tag="ctx")
                    for p in range(PAIRS):
                        nc.tensor.matmul(
                            ctx_ps[:, p * P:(p + 1) * P],
                            lhsT=kv[:, C + p * P: C + (p + 1) * P],   # v pair
                            rhs=kv[:, p * P:(p + 1) * P],             # k pair
                            start=True, stop=True,
                        )
                    if nt == 0:
                        nc.vector.tensor_copy(ctx_acc, ctx_ps)
                    else:
                        nc.vector.tensor_add(ctx_acc, ctx_acc, ctx_ps)

                def fused_out(xck, cidx):
                    ctx_bd_o, o_r = fuse
                    for p in range(PAIRS):
                        out_ps = ps_out.tile([P, 512], F32, tag="psout")
                        nc.tensor.matmul(out_ps, lhsT=ctx_bd_o[:, p, :],
                                         rhs=xck[:, p, :], start=True, stop=True)
                        stg = outst.tile([P, 512], BF16, tag="stg")
                        nc.vector.tensor_add(stg, out_ps, xck[:, p, :])
                        nc.scalar.dma_start(
                            out=o_r[:, p, cidx * 512:(cidx + 1) * 512], in_=stg)

                prev = None
                xck = None
                for nt in range(NT):
                    if nt % 4 == 0:
                        cs = (nt // 4) * 512
                        xck = xsp.tile([P, KT, 512], BF16, tag="xs")
                        nc.sync.dma_start(out=xck, in_=src_r[:, :, cs:cs + 512])
                        if fuse is not None:
                            fused_out(xck, nt // 4)
                    kv = kvsb.tile([P, 2 * C], BF16, tag="kv")
                    for ch in range(4):
                        kv_ps = ps_kv.tile([P, 512], F32, tag="kvps")
                        for kt in range(KT):
                            nc.tensor.matmul(
                                kv_ps,
                                lhsT=xck[:, kt, (nt % 4) * P:(nt % 4 + 1) * P],
                                rhs=W[:, kt, kvcol0 + ch * 512: kvcol0 + (ch + 1) * 512],
                                start=(kt == 0), stop=(kt == KT - 1),
                            )
                        nc.scalar.copy(kv[:, ch * 512:(ch + 1) * 512], kv_ps)
                    if prev is not None:
                        pair_mms(*prev)
                    prev = (kv, nt)
                pair_mms(*prev)
                return ctx_acc

            def softmax_pair(ctx_sb, p, ctx_bd):
                """Softmax over d (free axis) of the two diag blocks of pair p, then
                PE-transpose into slice p of the block-diagonal bf16 ctx tile."""
                S = smaxp.tile([P, P], F32, tag="smax")
                nc.vector.memset(S, 0.0)
                for r0 in (0, 64):
                    blk = ctx_sb[r0:r0 + 64, p * P + r0: p * P + r0 + 64]
                    mx = stats.tile([P, 1], F32, tag="mx")
                    nc.vector.reduce_max(mx[r0:r0 + 64], blk, axis=mybir.AxisListType.X)
                    ng = stats.tile([P, 1], F32, tag="ng")
                    nc.scalar.mul(ng[r0:r0 + 64], mx[r0:r0 + 64], -SCALE)
                    se = stats.tile([P, 1], F32, tag="se")
                    nc.scalar.activation(
                        S[r0:r0 + 64, r0:r0 + 64], blk,
                        mybir.ActivationFunctionType.Exp,
                        bias=ng[r0:r0 + 64], scale=SCALE,
                        accum_out=se[r0:r0 + 64],
                    )
                    rv = stats.tile([P, 1], F32, tag="rv")
                    nc.vector.reciprocal(rv[r0:r0 + 64], se[r0:r0 + 64])
                    nc.vector.tensor_scalar_mul(
                        S[r0:r0 + 64, r0:r0 + 64], S[r0:r0 + 64, r0:r0 + 64],
                        rv[r0:r0 + 64],
                    )
                tr_ps = ps_out.tile([P, P], F32, tag="psout")
                nc.tensor.transpose(tr_ps, S, ident)
                nc.vector.tensor_copy(ctx_bd[:, p, :], tr_ps)

            def q_out_phase(xT_src_r, W, ctx_bd, spill_r):
                """Second self pass: stream x^T chunks; per chunk compute all
                pairs' q, the ctx product, and the residual; spill x' to DRAM."""
                for ch in range(CH):
                    xck = xsp.tile([P, KT, 512], BF16, tag="xs")
                    nc.scalar.dma_start(out=xck,
                                        in_=xT_src_r[:, :, ch * 512:(ch + 1) * 512])
                    for p in range(PAIRS):
                        qt_ps = ps_qt.tile([P, 512], F32, tag="qt")
                        for kt in range(KT):
                            nc.tensor.matmul(
                                qt_ps,
                                lhsT=W[:, kt, p * P:(p + 1) * P],
                                rhs=xck[:, kt, :],
                                start=(kt == 0), stop=(kt == KT - 1),
                            )
                        qts = qtsp.tile([P, 512], BF16, tag="qts")
                        nc.scalar.copy(qts, qt_ps)
                        out_ps = ps_out.tile([P, 512], F32, tag="psout")
                        nc.tensor.matmul(out_ps, lhsT=ctx_bd[:, p, :], rhs=qts,
                                         start=True, stop=True)
                        stg = outst.tile([P, 512], BF16, tag="stg")
                        nc.vector.tensor_add(stg, out_ps, xck[:, p, :])
                        nc.sync.dma_start(
                            out=spill_r[:, p, ch * 512:(ch + 1) * 512], in_=stg)

            def cross_out(o_r, ctx_bd, q_src_r):
                """o = merge(q @ ctx) + residual; q rows streamed per (pair, chunk)."""
                for p in range(PAIRS):
                    for ch in range(CH):
                        qrow = qrowp.tile([P, 512], BF16, tag="qrow")
                        nc.sync.dma_start(
                            out=qrow, in_=q_src_r[:, p, ch * 512:(ch + 1) * 512])
                        out_ps = ps_out.tile([P, 512], F32, tag="psout")
                        nc.tensor.matmul(out_ps, lhsT=ctx_bd[:, p, :], rhs=qrow,
                                         start=True, stop=True)
                        stg = outst.tile([P, 512], BF16, tag="stg")
                        nc.vector.tensor_add(stg, out_ps, qrow)
                        nc.scalar.dma_start(
                            out=o_r[:, p, ch * 512:(ch + 1) * 512], in_=stg)

            def self_stage(xN, xT_src_r, W, spill_r):
                ctx_sb = gram_ctx(xN, W)
                ctx_bd = ctxsb.tile([P, PAIRS, P], BF16, tag="ctx_bd")
                for p in range(PAIRS):
                    softmax_pair(ctx_sb, p, ctx_bd)
                q_out_phase(xT_src_r, W, ctx_bd, spill_r)

            def load_xn(dst, src_r):
                for g in range(0, NT, 4):
                    nc.scalar.dma_start(out=dst[:, g:g + 4, :], in_=src_r[:, g:g + 4, :])

            with (tc.For_i(0, niter_v, 1, name="rep") if loop else _nullctx()):
                # ---- self stage, stream 1 ----
                # Gram only needs token-major x, so its chunks queue first; the
                # weights follow in use order (Wv for BT, Wk for pairs, Wq last).
                x1n = xnp.tile([P, NT, C], BF16, tag="xn")
                load_xn(x1n, x1N_r)
                Wsq = wts.tile([P, KT, 3 * C], BF16, tag="wts")
                for c0 in [2 * C, 2 * C + 512, C, C + 512, 0, 512]:
                    nc.scalar.dma_start(out=Wsq[:, :, c0:c0 + 512],
                                        in_=Wsq_r[:, :, c0:c0 + 512])
                self_stage(x1n, x1T_r, Wsq, x1p_r)

                # ---- self stage, stream 2 (x2N loads after Gram-1 frees slot) ----
                x2n = xnp.tile([P, NT, C], BF16, tag="xn")
                load_xn(x2n, x2N_r)
                self_stage(x2n, x2T_r, Wsq, x2p_r)

                # ---- cross stage ----
                Wkv2 = wts.tile([P, KT, 2 * C], BF16, tag="wts")
                for c0 in (0, 512, 1024, 1536):
                    nc.scalar.dma_start(out=Wkv2[:, :, c0:c0 + 512],
                                        in_=Wkv2_r[:, :, c0:c0 + 512])
                ctx2_sb = ctx_accumulate_s(x2p_r, Wkv2, kvcol0=0)
                ctx2_bd = ctxsb.tile([P, PAIRS, P], BF16, tag="ctx_bd")
                for p in range(PAIRS):
                    softmax_pair(ctx2_sb, p, ctx2_bd)

                Wkv1 = wts.tile([P, KT, 2 * C], BF16, tag="wts")
                for c0 in (0, 512, 1024, 1536):
                    nc.scalar.dma_start(out=Wkv1[:, :, c0:c0 + 512],
                                        in_=Wkv1_r[:, :, c0:c0 + 512])
                # ctx1 streams x1p; o1 = q1 @ ctx2 + x1' is fused into the same
                # stream (q1 rows are slices of each resident chunk).
                ctx1_sb = ctx_accumulate_s(x1p_r, Wkv1, kvcol0=0,
                                           fuse=(ctx2_bd, o1T_r))
                ctx1_bd = ctxsb.tile([P, PAIRS, P], BF16, tag="ctx_bd")
                for p in range(PAIRS):
                    softmax_pair(ctx1_sb, p, ctx1_bd)
                cross_out(o2T_r, ctx1_bd, x2p_r)                # o2 = q2 @ ctx1 + x2'

    nc.finalize()
    return nc


def _get_nc():
    if "nc" not in _CACHE:
        _CACHE["nc"] = _build()
    return _CACHE["nc"]


def make_in_maps(x1, x2, Wsqkv1, Wkv1, Wkv2, niter=1):
    x1 = np.asarray(x1, dtype=np.float32)
    x2 = np.asarray(x2, dtype=np.float32)
    Wsq_b = np.ascontiguousarray(np.asarray(Wsqkv1, np.float32)).astype(ml_dtypes.bfloat16)
    Wkv1_b = np.ascontiguousarray(np.asarray(Wkv1, np.float32)).astype(ml_dtypes.bfloat16)
    Wkv2_b = np.ascontiguousarray(np.asarray(Wkv2, np.float32)).astype(ml_dtypes.bfloat16)
    nit = np.array([[niter]], dtype=np.int32)
    in_maps = []
    for b in range(B):
        x1b = x1[b].astype(ml_dtypes.bfloat16)
        x2b = x2[b].astype(ml_dtypes.bfloat16)
        in_maps.append({
            "niter": nit,
            "x1T": np.ascontiguousarray(x1b.T),
            "x2T": np.ascontiguousarray(x2b.T),
            "x1N": x1b,
            "x2N": x2b,
            "Wsqkv": Wsq_b,
            "Wkv1": Wkv1_b,
            "Wkv2": Wkv2_b,
        })
    return in_maps


def gather_outputs(results):
    o1 = np.stack([np.asarray(results[b]["o1T"]).astype(np.float32).T for b in range(B)])
    o2 = np.stack([np.asarray(results[b]["o2T"]).astype(np.float32).T for b in range(B)])
    return o1, o2


def kernel(x1, x2, Wsqkv1, Wkv1, Wkv2, num_heads=16, selfattn=1, **_unused):
    in_maps = make_in_maps(x1, x2, Wsqkv1, Wkv1, Wkv2)
    nc = _get_nc()
    res = run_bass_kernel_spmd(nc, in_maps, core_ids=list(range(B)),
                               trace=bool(int(os.environ.get("KERNEL_TRACE", "0"))))
    _CACHE["last_result"] = res
    return gather_outputs(res.results)
